# revision 1
# baseline (speedup 1.0000x reference)
"""Differentiable-stack kernel for Trainium2 (Bass/Tile), 8-core data parallel.

The reference "soft stack" only ever reads slot S-1, and the shift moves
slot s+1 -> slot s (never upward), so slot S-1 receives only `val`.  The
output therefore reduces to a gated linear recurrence per (batch, d):

    h_t = (1-o_t) * ((1-p_t) * h_{t-1} + p_t * x_t)
        = a_t * h_{t-1} + b_t * x_t
    a_t = (1-p_t)(1-o_t),  b_t = p_t (1-o_t)       (scalars per (b, t))

Device strategy (per core: 2 batch elements of [L=2048, D=512] f32):
  partitions p = (batch, chunk) with 64 chunks of K=32 steps -> 128 parts,
  free dim = (k, d-block).  Three passes over the data:
    1) c_k = b_k * x_k           (ScalarE activation, per-partition scale)
    2) chain: y_k = a_k*y_{k-1} + c_k  (31 seq. DVE scalar_tensor_tensor)
    3) fixup: y_k += r_k * H     (DVE/GPSIMD split; r = cumprod(a))
  The cross-chunk carry H is exact: a shifted-identity+scan builds the
  chunk-to-chunk decay matrix W (products accumulated sequentially, no
  logs/divisions), then H = W^T.T @ y_last via one PE matmul per D-block.
"""

import os
from contextlib import ExitStack

import numpy as np

import concourse.bass as bass
import concourse.tile as tile
from concourse import bacc, mybir
from concourse.bass_utils import run_bass_kernel_spmd

F32 = mybir.dt.float32
ALU = mybir.AluOpType
ACTF = mybir.ActivationFunctionType

B, L, D = 16, 2048, 512
NCORES = 8
BPC = B // NCORES            # batches per core = 2
C = 64                       # chunks per batch element
K = L // C                   # timesteps per chunk = 32
P = BPC * C                  # SBUF partitions = 128

NJOB = int(os.environ.get("DSTACK_NJOB", "2"))
DBLK = D // NJOB
LSPLIT = int(os.environ.get("DSTACK_LSPLIT", "4"))  # DMA splits along k
XBUFS = int(os.environ.get("DSTACK_XBUFS", "1"))
E = int(os.environ.get("DSTACK_WIN", "16"))         # rescale window length
NW = K // E
YPSUM_BUFS = int(os.environ.get("DSTACK_YPSUM", "4"))
# per-k engine for the z+F add: g=gpsimd TT, t=tensorE matmul-pair, v=DVE TT
# (per job, comma separated)
ADD_PATTERNS = os.environ.get("DSTACK_ADDS", "tg,tgv").split(",")
# per-k engine for the rho scale: a=scalarE activation, v=DVE tensor_scalar
SCALE_PATTERNS = os.environ.get("DSTACK_SCALES", "a,av").split(",")
STQ = os.environ.get("DSTACK_STQ", "sync")  # store DMA issue queue


def build_module():
    # Bacc (not plain Bass): its compile() pass splits multi-sem waits into
    # event-semaphore instructions — TRN2 instructions have 1 wait slot.
    nc = bacc.Bacc("TRN2", target_bir_lowering=False)
    xin = nc.dram_tensor("xin", [P, K * D], F32, kind="ExternalInput")
    pg = nc.dram_tensor("pg", [P, K], F32, kind="ExternalInput")
    og = nc.dram_tensor("og", [P, K], F32, kind="ExternalInput")
    yout = nc.dram_tensor("yout", [P, K * D], F32, kind="ExternalOutput")

    x3 = xin[:].rearrange("p (k d) -> p k d", k=K)
    y3 = yout[:].rearrange("p (k d) -> p k d", k=K)

    with tile.TileContext(nc) as tc, ExitStack() as ctx:
        smalls = ctx.enter_context(tc.tile_pool(name="smalls", bufs=1))
        xpool = ctx.enter_context(tc.tile_pool(name="xpool", bufs=XBUFS))
        hpool = ctx.enter_context(tc.tile_pool(name="hpool", bufs=2))
        pspool = ctx.enter_context(tc.tile_pool(name="pspool", bufs=2, space="PSUM"))

        # ---------------- kick off all input DMAs first ----------------
        pgt = smalls.tile([P, K], F32)
        ogt = smalls.tile([P, K], F32)
        nc.sync.dma_start(pgt[:], pg[:])
        nc.sync.dma_start(ogt[:], og[:])

        # One shared full-D tile: k-major loads are fully contiguous per
        # partition (16KB runs -> few descriptors, full DMA bandwidth); the
        # D-block jobs then compute on d-slices of it.
        ksp = K // LSPLIT
        xt_full = xpool.tile([P, K, D], F32)
        for s in range(LSPLIT):
            nc.sync.dma_start(
                xt_full[:, s * ksp:(s + 1) * ksp, :],
                x3[:, s * ksp:(s + 1) * ksp, :])

        # ---------------- gate preprocessing (tiny) ----------------

        om1 = smalls.tile([P, K], F32)
        av = smalls.tile([P, K], F32)
        bv = smalls.tile([P, K], F32)
        # om1 = 1 - o ; av = (1-p)(1-o) ; bv = p(1-o)
        nc.vector.tensor_scalar(om1[:], ogt[:], -1.0, 1.0, ALU.mult, ALU.add)
        nc.vector.tensor_scalar(av[:], pgt[:], -1.0, 1.0, ALU.mult, ALU.add)
        nc.vector.tensor_mul(av[:], av[:], om1[:])
        nc.vector.tensor_mul(bv[:], pgt[:], om1[:])

        # r = inclusive cumprod of a along k (exact, sequential products)
        zk = smalls.tile([P, K], F32)
        nc.vector.memset(zk[:], 0.0)
        rv = smalls.tile([P, K], F32)
        nc.vector.tensor_tensor_scan(rv[:], av[:], zk[:], 1.0, ALU.mult, ALU.add)

        # ---------------- cross-chunk decay matrix W ----------------
        # R[c] = rv[c, K-1] (per-chunk total decay).  Build
        #   WT[c', j] = prod_{u=c'+1}^{j-1} R_u   for c' < j in same batch
        # via one scan over a broadcast shifted-R row with shifted-identity
        # injections.  H_j = sum_{c'} WT[c', j] * y_last[c'] is the carry
        # into chunk j.
        # PE Matmult supports a single inline sync wait, so every PE input
        # must have DVE as its last writer: copy the gpsimd-built identity
        # through DVE, and do the rsh shift-copy on DVE.
        ident = smalls.tile([P, P], F32)
        nc.gpsimd.memset(ident[:], 0.0)
        nc.gpsimd.affine_select(
            out=ident[:], in_=ident[:], compare_op=ALU.not_equal, fill=1.0,
            base=0, pattern=[[-1, P]], channel_multiplier=1)
        identv = smalls.tile([P, P], F32)
        nc.vector.tensor_copy(identv[:], ident[:])

        rrow_ps = pspool.tile([1, P], F32, bufs=1)
        nc.tensor.transpose(rrow_ps[:], rv[:, K - 1:K], identv[:])

        rsh = smalls.tile([1, P], F32)
        nc.vector.memset(rsh[:], 0.0)
        nc.vector.tensor_copy(rsh[0:1, 1:P], rrow_ps[0:1, 0:P - 1])
        # zero the batch boundary so batches don't mix
        nc.vector.memset(rsh[0:1, C:C + 1], 0.0)

        # broadcast rsh row across all partitions: rank-1 PE matmul ones^T @ rsh
        ones1p = smalls.tile([1, P], F32)
        nc.vector.memset(ones1p[:], 1.0)
        rb = pspool.tile([P, P], F32, bufs=1)
        nc.tensor.matmul(rb[:], ones1p[:], rsh[:], start=True, stop=True)

        ish = smalls.tile([P, P], F32)
        nc.gpsimd.memset(ish[:], 0.0)
        nc.gpsimd.affine_select(
            out=ish[:], in_=ish[:], compare_op=ALU.not_equal, fill=1.0,
            base=-1, pattern=[[1, P]], channel_multiplier=-1)
        # last chunk of each batch feeds nothing within its batch: zero row
        # C-1 (row P-1 is already zero since j==P does not exist).  Engine ops
        # cannot start at partition 63, so use a per-partition mask multiply.
        mask_col = smalls.tile([P, 1], F32)
        nc.gpsimd.memset(mask_col[:], 1.0)
        nc.gpsimd.affine_select(
            out=mask_col[:], in_=mask_col[:], compare_op=ALU.not_equal, fill=0.0,
            base=-(C - 1), pattern=[[1, 1]], channel_multiplier=1)
        nc.vector.tensor_scalar(ish[:], ish[:], mask_col[:], None, ALU.mult)

        wt = smalls.tile([P, P], F32)
        nc.vector.tensor_tensor_scan(wt[:], rb[:], ish[:], 0.0, ALU.mult, ALU.add)

        # ---------------- windowed rescaling gates ----------------
        # Within windows of E steps the recurrence is computed as a rescaled
        # prefix sum:  z_k = (b_k/rho_k) x_k + z_{k-1},  y_k = rho_k (z_k + F_w)
        # where rho is the window-local inclusive cumprod of a (restarts every
        # E steps keep b/rho in fp32 range) and F_w folds the window carry and
        # the global chunk carry H.
        awin0 = smalls.tile([P, K], F32)
        nc.vector.tensor_copy(awin0[:], av[:])
        a0v = awin0[:].rearrange("p (w e) -> p w e", e=E)
        nc.vector.memset(a0v[:, :, 0:1], 0.0)
        awin1 = smalls.tile([P, K], F32)
        nc.vector.memset(awin1[:], 0.0)
        a1v = awin1[:].rearrange("p (w e) -> p w e", e=E)
        avv = av[:].rearrange("p (w e) -> p w e", e=E)
        nc.vector.tensor_copy(a1v[:, :, 0:1], avv[:, :, 0:1])
        rho = smalls.tile([P, K], F32)
        nc.vector.tensor_tensor_scan(rho[:], awin0[:], awin1[:], 0.0,
                                     ALU.mult, ALU.add)

        rcp = smalls.tile([P, K], F32)
        nc.vector.reciprocal(rcp[:], rho[:])
        wv = smalls.tile([P, K], F32)
        nc.vector.tensor_mul(wv[:], bv[:], rcp[:])

        # ACT-written copies of rho / w for ScalarE ops (single-wait encoding:
        # the AP-scale ops then only wait on their data input), plus a tiny
        # copy to absorb the ACT same-engine completion wait.
        rhoa = smalls.tile([P, K], F32)
        nc.scalar.copy(rhoa[:], rho[:])
        wva = smalls.tile([P, K], F32)
        nc.scalar.copy(wva[:], wv[:])
        scrap = smalls.tile([1, 1], F32)
        nc.scalar.copy(scrap[:], rhoa[0:1, 0:1])

        # ---------------- main loop over D blocks ----------------
        # Phase 1 for ALL jobs first (keeps the DVE chain stream dense),
        # then per-job carry matmul + finals.  In-order engine queues make
        # emission order matter: H/F for job j are emitted before job j+1's
        # finals so PE never blocks ready work.
        def chains(j):
            xt = xt_full[:, :, j * DBLK:(j + 1) * DBLK]
            for w in range(NW):
                k0 = w * E
                nc.scalar.activation(
                    xt[:, k0, :], xt[:, k0, :], ACTF.Copy,
                    bias=0.0, scale=wva[:, k0:k0 + 1])
                for e in range(1, E):
                    k = k0 + e
                    nc.vector.scalar_tensor_tensor(
                        xt[:, k, :], xt[:, k, :], wv[:, k:k + 1],
                        xt[:, k - 1, :], ALU.mult, ALU.add)
            # window carries (y-domain): c_{w+1} = rho_last (c_w + z_last)
            carries = [None]
            cprev = None
            for w in range(1, NW + 1):
                klast = w * E - 1
                if cprev is None:
                    cw = hpool.tile([P, DBLK], F32, tag=f"cw{w}_{j}")
                    nc.vector.tensor_scalar(
                        cw[:], xt[:, klast, :], rho[:, klast:klast + 1], None,
                        ALU.mult)
                else:
                    tadd = hpool.tile([P, DBLK], F32, tag=f"tadd{j}")
                    nc.vector.tensor_add(tadd[:], cprev[:], xt[:, klast, :])
                    cw = hpool.tile([P, DBLK], F32, tag=f"cw{w}_{j}")
                    nc.vector.tensor_scalar(
                        cw[:], tadd[:], rho[:, klast:klast + 1], None,
                        ALU.mult)
                carries.append(cw)
                cprev = cw
            return carries

        def carry_and_finals(j, carries):
            xt = xt_full[:, :, j * DBLK:(j + 1) * DBLK]
            d0 = j * DBLK
            yll = carries[NW]
            # chunk carry: H = WT.T @ y_ll
            hps = pspool.tile([P, DBLK], F32, tag=f"hps{j}", bufs=1)
            nc.tensor.matmul(hps[:], wt[:], yll[:], start=True, stop=True)
            hs = hpool.tile([P, DBLK], F32, tag=f"hs{j}")
            nc.scalar.copy(hs[:], hps[:])
            # F_w = c_w + r_{wE-1} * H  (one DVE op per extra window)
            fws = [hs]
            for w in range(1, NW):
                fw = hpool.tile([P, DBLK], F32, tag=f"fw{w}_{j}")
                nc.vector.scalar_tensor_tensor(
                    fw[:], hs[:], rv[:, w * E - 1:w * E], carries[w][:],
                    ALU.mult, ALU.add)
                fws.append(fw)

            # finals: y_k = rho_k * (z_k + F_w)
            adds = ADD_PATTERNS[j % len(ADD_PATTERNS)]
            scls = SCALE_PATTERNS[j % len(SCALE_PATTERNS)]
            for k in range(K):
                w = k // E
                amode = adds[k % len(adds)]
                smode = scls[k % len(scls)]
                if amode == "t":
                    yps = pspool.tile([P, DBLK], F32, tag="ypsum",
                                      bufs=YPSUM_BUFS)
                    nc.tensor.matmul(yps[:], identv[:], xt[:, k, :],
                                     start=True, stop=False)
                    nc.tensor.matmul(yps[:], identv[:], fws[w][:],
                                     start=False, stop=True)
                    src = yps[:]
                else:
                    eng = nc.vector if amode == "v" else nc.gpsimd
                    eng.tensor_add(xt[:, k, :], xt[:, k, :], fws[w][:])
                    src = xt[:, k, :]
                if smode == "a":
                    nc.scalar.activation(
                        xt[:, k, :], src, ACTF.Copy,
                        bias=0.0, scale=rhoa[:, k:k + 1])
                else:
                    nc.vector.tensor_scalar(
                        xt[:, k, :], src, rho[:, k:k + 1], None, ALU.mult)

            st_eng = getattr(nc, STQ)
            for s in range(LSPLIT):
                st_eng.dma_start(
                    y3[:, s * ksp:(s + 1) * ksp, d0:d0 + DBLK],
                    xt[:, s * ksp:(s + 1) * ksp, :])

        if os.environ.get("DSTACK_PHASED", "0") == "1":
            all_carries = [chains(j) for j in range(NJOB)]
            for j in range(NJOB):
                carry_and_finals(j, all_carries[j])
        else:
            for j in range(NJOB):
                carry_and_finals(j, chains(j))

    nc.compile()
    return nc


_module_cache = {}


def _get_module():
    if "nc" not in _module_cache:
        _module_cache["nc"] = build_module()
    return _module_cache["nc"]


def make_in_maps(x, push_gate, pop_gate):
    x = np.ascontiguousarray(np.asarray(x), dtype=np.float32)
    pgf = np.ascontiguousarray(np.asarray(push_gate), dtype=np.float32).reshape(B, L)
    ogf = np.ascontiguousarray(np.asarray(pop_gate), dtype=np.float32).reshape(B, L)
    in_maps = []
    for i in range(NCORES):
        sl = slice(i * BPC, (i + 1) * BPC)
        in_maps.append({
            "xin": np.ascontiguousarray(x[sl].reshape(P, K * D)),
            "pg": np.ascontiguousarray(pgf[sl].reshape(P, K)),
            "og": np.ascontiguousarray(ogf[sl].reshape(P, K)),
        })
    return in_maps


def run(x, push_gate, pop_gate, **spmd_kwargs):
    """Run on hardware; returns (output, BassKernelResults)."""
    nc = _get_module()
    in_maps = make_in_maps(x, push_gate, pop_gate)
    res = run_bass_kernel_spmd(nc, in_maps, core_ids=list(range(NCORES)),
                               **spmd_kwargs)
    out = np.concatenate(
        [res.results[i]["yout"].reshape(BPC, L, D) for i in range(NCORES)],
        axis=0)
    return out, res


def kernel(x, push_gate, pop_gate):
    out, _ = run(x, push_gate, pop_gate)
    return out

